# revision 14
# baseline (speedup 1.0000x reference)
"""ChannelAttentionPropagation1D kernel for 8x TRN2 NeuronCores.

Reference computation (per batch b):
  kv[c,d]   = sum_{t,n} key_mem[b,t,n,c] * val_mem[b,t,n,d]    # (64, 64)
  kv_soft   = softmax(kv, axis=c)
  out[n,d]  = alpha * (key_cur[b] @ kv_soft)[n,d] + val_cur[b,n,d]

Sharding (8 cores):
  phase 1: core i contracts the t=i slice of key_mem/val_mem (16384 tokens
           per batch) into a partial kv^T, AllGathered over cores in two
           2-batch groups.
  phase 2: core i computes the n-slice [2048*i, 2048*(i+1)) of the output.

Precision: key_mem/val_mem are cast to fp8e4m3 and key_cur to fp16 on the
host; the output is stored fp16 and upcast on the host. Empirical rel-fro
error on the reference data is 6.8e-3, under the 2e-2 gate: the kv logits
have std ~600 so the softmax is near-one-hot, the fp8 matmuls accumulate
exactly in fp32 PSUM (4-bit mantissa products are exact), and the quant
noise on the logits (std ~30) flips an argmax only where the top-2 gap is
already tiny. fp8 halves the dominant HBM stream again vs fp16 and enables
DoubleRow matmuls (256-token contraction per PE instruction).

Layout notes:
  - phase 1 accumulates kvT[d,c] (PSUM) so the softmax axis c lands on the
    free axis; a tiny PE transpose afterwards yields kv_soft[c,d] (fp16).
  - all DRAM operands are host-packed to the exact SBUF layout so every DMA
    is a dense [128, rowbytes] block copy.
  - key_cur is transposed (and scaled by alpha) on the host so its channel
    axis is the SBUF partition axis; its token axis is permuted n = 16p + j
    so phase-2 output tiles assemble into contiguous-per-partition stores.
  - collectives: the framework barrier (device rendezvous) ends at
    ~skew+preamble; the CC core then serializes AllGathers at ~11us
    dequeue + transfer each. Two group AGs let the first one's transfer
    overlap the second half of phase 1, and group-major tails let the
    first group's phase 2 overlap the second AllGather.
"""

import numpy as np
import ml_dtypes

import concourse.bacc as bacc
import concourse.mybir as mybir
import concourse.tile as tile
from concourse import bass_utils, masks

F32 = mybir.dt.float32
F16 = mybir.dt.float16
F8 = mybir.dt.float8e4

N_CORES = 8
N, T, NTOK, C, C2 = 4, 8, 16384, 64, 64
NSL = NTOK // N_CORES  # 2048: phase-2 token slice per core
A_TILES = 64           # 128-token matmul tiles per half-batch chunk
HALF = NTOK // 2       # 8192 tokens per phase-1 DMA chunk

_CACHE = {}

# Extra kwargs forwarded to run_bass_kernel_spmd (used by the profiling
# harness to request an NTFF trace; empty for normal correctness runs).
_RUN_OPTS = {}


def _build_program():
    nc = bacc.Bacc(
        "TRN2",
        target_bir_lowering=False,
        debug=False,
        enable_asserts=False,
        num_devices=N_CORES,
    )

    # host-packed [b, h, p, a, c]: token t = h*8192 + p*64 + a
    km = nc.dram_tensor(
        "key_mem", [N, 2, 128, A_TILES * C], F8, kind="ExternalInput"
    ).ap()
    vm = nc.dram_tensor(
        "val_mem", [N, 2, 128, A_TILES * C2], F8, kind="ExternalInput"
    ).ap()
    # key_curT is host-packed [128, NSL/2]: rows 0:64 = channels for output
    # tiles j=0..7, rows 64:128 = channels for tiles j=8..15 (row-tiled
    # phase-2 pairs).
    kct = nc.dram_tensor(
        "key_curT", [N, 128, NSL // 2], F16, kind="ExternalInput"
    ).ap()
    vc = nc.dram_tensor("val_cur", [N, NSL, C2], F16, kind="ExternalInput").ap()
    # output is stored transposed-and-packed [b, (half,d), (jm,p)] and
    # un-permuted on the host (free); this lets phase 2 run with the
    # 512-wide moving operand (4 matmuls per batch instead of 16)
    out = nc.dram_tensor(
        "out", [N, 128, (NSL // 2)], F16, kind="ExternalOutput"
    ).ap()

    with tile.TileContext(nc) as tc:
        with (
            tc.tile_pool(name="persist", bufs=1) as persist,
            tc.tile_pool(name="big", bufs=4) as big,
            tc.tile_pool(name="tmp", bufs=2) as tmp,
            tc.tile_pool(name="stage", bufs=2) as stage_pool,
            tc.tile_pool(name="ps", bufs=2, space="PSUM") as ps,
            tc.tile_pool(name="dram", bufs=1, space="DRAM") as dram,
        ):
            ident = persist.tile([128, 128], F32)
            masks.make_identity(nc, ident[:])

            kct_sb = persist.tile([128, N * (NSL // 2)], F16)
            vc_sb = persist.tile([128, N * (NSL // 128) * C2], F16)

            kvt_sb = persist.tile([C2, N * C], F16)
            # merged AllGather result [d, (rank, N*C)], fp16 to halve
            # the collective payload and readback (partials are ~+-1500
            # with quant noise ~30 from fp8, so fp16 is free)
            kvt_all = persist.tile([C2, N_CORES * N * C], F16)
            kvt_red = persist.tile([C2, N * C], F16)
            kv_soft = persist.tile([128, N * C2], F16)
            ar_outs = {}

            def emit_tail():
                """AR readback + tree-reduce + softmax + transpose + phase 2
                + stores for all 4 batches, emitted after the whole phase 1
                so a late peer can never block local phase-1 work (engine
                FIFOs run in program order)."""
                RW = N * C  # 256: per-rank width in kvt_all
                nc.sync.dma_start(
                    kvt_all[:].rearrange("d (r c) -> d r c", r=N_CORES),
                    ar_outs[0].rearrange("r d c -> d r c"),
                )
                # tree-reduce the 8 ranks: widths 1024/512, then final 256
                # into kvt_red (cols b*C of kvt_red match batch b)
                for width in (4 * RW, 2 * RW):
                    nc.vector.tensor_add(
                        kvt_all[:, 0:width],
                        kvt_all[:, 0:width],
                        kvt_all[:, width:2 * width],
                    )
                nc.vector.tensor_add(
                    kvt_red[:], kvt_all[:, 0:RW], kvt_all[:, RW:2 * RW]
                )
                neg_mx = tmp.tile([C2, N], F16, tag="mx", name="mx")
                ex = tmp.tile([C2, N * C], F32, tag="ex", name="ex")
                sm = tmp.tile([C2, N], F32, tag="sm", name="sm")
                rv = tmp.tile([C2, N], F32, tag="rv", name="rv")
                for i, b in enumerate(range(N)):
                    nc.vector.reduce_max(
                        out=neg_mx[:, i:i + 1],
                        in_=kvt_red[:, b * C:(b + 1) * C],
                        axis=mybir.AxisListType.X,
                        negate=True,
                    )
                    nc.scalar.activation(
                        ex[:, i * C:(i + 1) * C],
                        kvt_red[:, b * C:(b + 1) * C],
                        mybir.ActivationFunctionType.Exp,
                        bias=neg_mx[:, i:i + 1], scale=1.0,
                        accum_out=sm[:, i:i + 1],
                    )
                    nc.vector.reciprocal(rv[:, i:i + 1], sm[:, i:i + 1])
                    nc.vector.tensor_scalar_mul(
                        ex[:, i * C:(i + 1) * C],
                        ex[:, i * C:(i + 1) * C],
                        rv[:, i:i + 1],
                    )
                    # Transpose softmaxed kvT to kv[c, d] (transpose-mode
                    # matmul must write PSUM partition 0); the PSUM->SBUF
                    # copies cast to fp16 for phase 2. The strip is copied
                    # twice — DVE writes partitions 0:64, ACT writes 64:128
                    # — so row-tiled phase-2 can read kv from the upper
                    # rows without a serializing SBUF->SBUF mirror DMA.
                    tp = ps.tile([C, C2], F32, tag="tp", name=f"tp{b}", bufs=2)
                    nc.tensor.transpose(
                        tp[:], ex[:, i * C:(i + 1) * C], ident[0:C2, 0:C2]
                    )
                    nc.vector.tensor_copy(
                        kv_soft[0:C, b * C2:(b + 1) * C2], tp[:]
                    )
                    nc.scalar.activation(
                        kv_soft[64:64 + C, b * C2:(b + 1) * C2],
                        tp[:],
                        mybir.ActivationFunctionType.Copy,
                    )
                # Phase 2, wide-moving form: out[d, n] = kv_soft^T @ kct
                # with kv_soft as the 64x64 stationary and kct as the moving
                # operand at the PE's max 512-wide free dim — 4 matmuls per
                # batch instead of 16 (the old form was LDWEIGHTS-dispatch
                # bound at ~151ns/instr). Row-half A (kct/kv rows 0:64,
                # tokens with n%16<8) runs on PE quadrant (0,0), half B on
                # (64,64) — concurrent subarrays — stacking both halves in
                # one [128, 512] psum bank per block. The transposed result
                # is stored as-is and un-permuted on the host.
                for b in range(N):
                    o_ps = ps.tile(
                        [128, 2, 512], F32, tag="o", name=f"o{b}", bufs=2
                    )
                    for blk in range(2):
                        for hf in range(2):
                            r0 = 64 * hf
                            nc.tensor.matmul(
                                o_ps[r0:r0 + C2, blk, :],
                                lhsT=kv_soft[r0:r0 + C, b * C2:(b + 1) * C2],
                                rhs=kct_sb[
                                    r0:r0 + C,
                                    b * 1024 + blk * 512: b * 1024 + (blk + 1) * 512,
                                ],
                                start=True,
                                stop=True,
                                tile_position=(r0, r0),
                            )
                    stg = stage_pool.tile(
                        [128, (NSL // 128) * C2], F16, tag=f"stg{b}",
                        name=f"stg{b}",
                    )
                    nc.vector.tensor_add(
                        stg[:, 0:512],
                        o_ps[:, 0, :],
                        vc_sb[:, b * 1024: b * 1024 + 512],
                    )
                    # store the first block while the second block's add runs
                    nc.sync.dma_start(out[b][:, 0:512], stg[:, 0:512])
                    nc.vector.tensor_add(
                        stg[:, 512:1024],
                        o_ps[:, 1, :],
                        vc_sb[:, b * 1024 + 512: b * 1024 + 1024],
                    )
                    nc.sync.dma_start(out[b][:, 512:1024], stg[:, 512:1024])

            # ---- phase 1: partial kvT[d, c] per batch, col-tiled 2x ----
            # Plain fp8 matmuls (NOT DoubleRow: with free dim 64 DoubleRow
            # disables fast-weight-load and is a measured net loss). Even
            # token-tiles accumulate on PE column group 0 (psum rows 0:64),
            # odd tiles on column group 2 (psum rows 64:128); the two
            # groups' LDWEIGHTS/MATMUL overlap on independent subarrays.
            for b in range(N):
                kv_ps = ps.tile([128, C], F32, tag="kv", name=f"kv{b}")
                for h in range(2):
                    k_sb = big.tile([128, A_TILES, C], F8, tag="k")
                    v_sb = big.tile([128, A_TILES, C2], F8, tag="v")
                    nc.sync.dma_start(
                        k_sb[:], km[b, h].rearrange("p (a c) -> p a c", a=A_TILES)
                    )
                    nc.sync.dma_start(
                        v_sb[:], vm[b, h].rearrange("p (a c) -> p a c", a=A_TILES)
                    )
                    if h == 1:
                        # phase-2 inputs for batch b: issued on the scalar
                        # (ACT) DMA FIFO so they never delay the phase-1
                        # chunk stream on the sync FIFO.
                        nc.scalar.dma_start(
                            kct_sb[:, b * (NSL // 2):(b + 1) * (NSL // 2)],
                            kct[b],
                        )
                        nc.scalar.dma_start(
                            vc_sb[:, b * 1024:(b + 1) * 1024],
                            vc[b].rearrange("(p j) c -> p (j c)", p=128),
                        )
                    for a in range(A_TILES):
                        half = a % 2
                        nc.tensor.matmul(
                            kv_ps[64 * half:64 * half + C2, :],
                            lhsT=v_sb[:, a, :],
                            rhs=k_sb[:, a, :],
                            start=(h == 0 and a < 2),
                            stop=(h == 1 and a >= A_TILES - 2),
                            tile_position=(0, 64 * half),
                        )
                # partial kvT = even-half + odd-half (DVE can read only one
                # PSUM operand per instruction, so copy then add)
                nc.vector.tensor_copy(kvt_sb[:, b * C:(b + 1) * C], kv_ps[0:C2, :])
                nc.vector.tensor_add(
                    kvt_sb[:, b * C:(b + 1) * C],
                    kvt_sb[:, b * C:(b + 1) * C],
                    kv_ps[64:64 + C2, :],
                )
            # ONE merged AllGather for all 4 batches (cheaper than
            # AllReduce on the CC core; the 8 partials are tree-reduced
            # locally on DVE). Phase 1 is short enough under fp8 that the
            # last partial is usually ready before the framework barrier
            # completes, so splitting the collective only adds the CC
            # core's ~9us-per-op serialization.
            ar_in = dram.tile([C2, N * C], F16, tag="ar_in", name="ar_in")
            ar_out = dram.tile(
                [N_CORES, C2, N * C], F16, addr_space="Shared",
                tag="ar_out", name="ar_out",
            )
            ar_outs[0] = ar_out
            nc.scalar.dma_start(ar_in[:], kvt_sb[:])
            nc.gpsimd.collective_compute(
                "AllGather",
                mybir.AluOpType.bypass,
                replica_groups=[list(range(N_CORES))],
                ins=[ar_in.opt()],
                outs=[ar_out.opt()],
            )
            emit_tail()

    nc.compile()
    return nc


def _get_program():
    if "nc" not in _CACHE:
        _CACHE["nc"] = _build_program()
    return _CACHE["nc"]


def kernel(key_mem, val_mem, key_cur, val_cur, alpha):
    key_mem = np.asarray(key_mem, dtype=np.float32)
    val_mem = np.asarray(val_mem, dtype=np.float32)
    key_cur = np.asarray(key_cur, dtype=np.float32)
    val_cur = np.asarray(val_cur, dtype=np.float32)
    alpha_f = float(np.asarray(alpha).reshape(-1)[0])

    nc = _get_program()

    # key_cur^T with alpha folded in; token axis permuted so that SBUF
    # column j*128+p holds token p*16+j (phase-2 store contiguity).
    kc_scaled = (alpha_f * key_cur).astype(np.float32)
    in_maps = []
    for i in range(N_CORES):
        kct_i = kc_scaled[:, i * NSL:(i + 1) * NSL, :].transpose(0, 2, 1)
        kct_i = (
            kct_i.reshape(N, C, 128, NSL // 128)
            .transpose(0, 1, 3, 2)
            .reshape(N, C, NSL)
        )
        # pack for row-tiled phase 2: rows 0:64 = tiles j=0..7,
        # rows 64:128 = tiles j=8..15
        kct_i = (
            kct_i.reshape(N, C, 2, NSL // 2)
            .transpose(0, 2, 1, 3)
            .reshape(N, 128, NSL // 2)
        )
        # phase-1 inputs: [b, h, p, a, c] with token t = h*8192 + p*64 + a
        # is a pure reshape of the [b, t, c] slice (p-major), so the DMA
        # sees dense 4KB partition rows.
        km_i = (
            key_mem[:, i]
            .reshape(N, 2, 128, A_TILES * C)
            .astype(ml_dtypes.float8_e4m3)
        )
        vm_i = (
            val_mem[:, i]
            .reshape(N, 2, 128, A_TILES * C2)
            .astype(ml_dtypes.float8_e4m3)
        )
        # val_cur packed to the phase-2 output layout [b, (half,d), (jm,p)]:
        # n = p*16 + half*8 + jm
        vc_i = val_cur[:, i * NSL:(i + 1) * NSL, :]
        vc_dn = (
            vc_i.reshape(N, 128, 2, 8, C2)
            .transpose(0, 2, 4, 3, 1)
            .reshape(N, 128, 1024)
        )
        in_maps.append(
            {
                "key_mem": np.ascontiguousarray(km_i),
                "val_mem": np.ascontiguousarray(vm_i),
                "key_curT": np.ascontiguousarray(kct_i.astype(np.float16)),
                "val_cur": np.ascontiguousarray(vc_dn.astype(np.float16)),
            }
        )

    res = bass_utils.run_bass_kernel_spmd(
        nc, in_maps, core_ids=list(range(N_CORES)), **_RUN_OPTS
    )
    _CACHE["last_result"] = res
    outs = []
    for i in range(N_CORES):
        # stored [b, (half,d), (jm,p)] -> [b, n, d] with n = p*16+half*8+jm
        st = res.results[i]["out"].reshape(N, 2, C2, 8, 128)
        outs.append(
            st.transpose(0, 4, 1, 3, 2).reshape(N, NSL, C2)
        )
    return np.concatenate(outs, axis=1).astype(np.float32)


# revision 15
# speedup vs baseline: 1.0001x; 1.0001x over previous
"""ChannelAttentionPropagation1D kernel for 8x TRN2 NeuronCores.

Reference computation (per batch b):
  kv[c,d]   = sum_{t,n} key_mem[b,t,n,c] * val_mem[b,t,n,d]    # (64, 64)
  kv_soft   = softmax(kv, axis=c)
  out[n,d]  = alpha * (key_cur[b] @ kv_soft)[n,d] + val_cur[b,n,d]

Sharding (8 cores):
  phase 1: core i contracts the t=i slice of key_mem/val_mem (16384 tokens
           per batch) into a partial kv^T, AllGathered over cores in two
           2-batch groups.
  phase 2: core i computes the n-slice [2048*i, 2048*(i+1)) of the output.

Precision: key_mem/val_mem are cast to fp8e4m3 and key_cur to fp16 on the
host; the output is stored fp16 and upcast on the host. Empirical rel-fro
error on the reference data is 6.8e-3, under the 2e-2 gate: the kv logits
have std ~600 so the softmax is near-one-hot, the fp8 matmuls accumulate
exactly in fp32 PSUM (4-bit mantissa products are exact), and the quant
noise on the logits (std ~30) flips an argmax only where the top-2 gap is
already tiny. fp8 halves the dominant HBM stream again vs fp16 and enables
DoubleRow matmuls (256-token contraction per PE instruction).

Layout notes:
  - phase 1 accumulates kvT[d,c] (PSUM) so the softmax axis c lands on the
    free axis; a tiny PE transpose afterwards yields kv_soft[c,d] (fp16).
  - all DRAM operands are host-packed to the exact SBUF layout so every DMA
    is a dense [128, rowbytes] block copy.
  - key_cur is transposed (and scaled by alpha) on the host so its channel
    axis is the SBUF partition axis; its token axis is permuted n = 16p + j
    so phase-2 output tiles assemble into contiguous-per-partition stores.
  - collectives: the framework barrier (device rendezvous) ends at
    ~skew+preamble; the CC core then serializes AllGathers at ~11us
    dequeue + transfer each. Two group AGs let the first one's transfer
    overlap the second half of phase 1, and group-major tails let the
    first group's phase 2 overlap the second AllGather.
"""

import numpy as np
import ml_dtypes

import concourse.bacc as bacc
import concourse.mybir as mybir
import concourse.tile as tile
from concourse import bass_utils, masks

F32 = mybir.dt.float32
F16 = mybir.dt.float16
F8 = mybir.dt.float8e4

N_CORES = 8
N, T, NTOK, C, C2 = 4, 8, 16384, 64, 64
NSL = NTOK // N_CORES  # 2048: phase-2 token slice per core
A_TILES = 64           # 128-token matmul tiles per half-batch chunk
HALF = NTOK // 2       # 8192 tokens per phase-1 DMA chunk

_CACHE = {}

# Extra kwargs forwarded to run_bass_kernel_spmd (used by the profiling
# harness to request an NTFF trace; empty for normal correctness runs).
_RUN_OPTS = {}


def _build_program():
    nc = bacc.Bacc(
        "TRN2",
        target_bir_lowering=False,
        debug=False,
        enable_asserts=False,
        num_devices=N_CORES,
    )

    # host-packed [b, h, p, a, c]: token t = h*8192 + p*64 + a
    km = nc.dram_tensor(
        "key_mem", [N, 2, 128, A_TILES * C], F8, kind="ExternalInput"
    ).ap()
    vm = nc.dram_tensor(
        "val_mem", [N, 2, 128, A_TILES * C2], F8, kind="ExternalInput"
    ).ap()
    # key_curT is host-packed [128, NSL/2]: rows 0:64 = channels for output
    # tiles j=0..7, rows 64:128 = channels for tiles j=8..15 (row-tiled
    # phase-2 pairs).
    kct = nc.dram_tensor(
        "key_curT", [N, 128, NSL // 2], F16, kind="ExternalInput"
    ).ap()
    vc = nc.dram_tensor("val_cur", [N, NSL, C2], F16, kind="ExternalInput").ap()
    # output is stored transposed-and-packed [b, (half,d), (jm,p)] and
    # un-permuted on the host (free); this lets phase 2 run with the
    # 512-wide moving operand (4 matmuls per batch instead of 16)
    out = nc.dram_tensor(
        "out", [N, 128, (NSL // 2)], F16, kind="ExternalOutput"
    ).ap()

    with tile.TileContext(nc) as tc:
        with (
            tc.tile_pool(name="persist", bufs=1) as persist,
            tc.tile_pool(name="big", bufs=4) as big,
            tc.tile_pool(name="tmp", bufs=2) as tmp,
            tc.tile_pool(name="stage", bufs=2) as stage_pool,
            tc.tile_pool(name="ps", bufs=2, space="PSUM") as ps,
            tc.tile_pool(name="dram", bufs=1, space="DRAM") as dram,
        ):
            ident = persist.tile([128, 128], F32)
            masks.make_identity(nc, ident[:])

            kct_sb = persist.tile([128, N * (NSL // 2)], F16)
            vc_sb = persist.tile([128, N * (NSL // 128) * C2], F16)

            kvt_sb = persist.tile([C2, N * C], F16)
            # AllGather results [d, (rank, group_width)], fp16 to halve
            # the collective payload and readback (partials are ~+-1500
            # with quant noise ~30 from fp8, so fp16 is free). Uneven
            # split: batches {0,1,2} gather first (their transfer and
            # tail overlap the second AG), batch {3} gathers alone so
            # the post-last-collective tail is minimal.
            kvt_allA = persist.tile([C2, N_CORES * 3 * C], F16)
            kvt_allB = persist.tile([C2, N_CORES * 1 * C], F16)
            kvt_red = persist.tile([C2, N * C], F16)
            kv_soft = persist.tile([128, N * C2], F16)
            ar_outs = {}

            def emit_tail(bs, g, kvt_all):
                """One group's AR readback + tree-reduce + softmax +
                transpose + phase 2 + stores, emitted after the whole
                phase 1 so a late peer can never block local phase-1 work
                (engine FIFOs run in program order). Group 0 ({0,1,2})
                overlaps group 1's tiny AllGather."""
                RW = len(bs) * C  # per-rank width in kvt_all
                nc.sync.dma_start(
                    kvt_all[:].rearrange("d (r c) -> d r c", r=N_CORES),
                    ar_outs[g].rearrange("r d c -> d r c"),
                )
                # tree-reduce the 8 ranks, then final add into kvt_red
                # (cols b*C of kvt_red match batch b)
                for width in (4 * RW, 2 * RW):
                    nc.vector.tensor_add(
                        kvt_all[:, 0:width],
                        kvt_all[:, 0:width],
                        kvt_all[:, width:2 * width],
                    )
                nc.vector.tensor_add(
                    kvt_red[:, bs[0] * C: bs[0] * C + RW],
                    kvt_all[:, 0:RW],
                    kvt_all[:, RW:2 * RW],
                )
                nb = len(bs)
                neg_mx = tmp.tile([C2, nb], F16, tag="mx", name=f"mx{g}")
                ex = tmp.tile([C2, nb * C], F32, tag="ex", name=f"ex{g}")
                sm = tmp.tile([C2, nb], F32, tag="sm", name=f"sm{g}")
                rv = tmp.tile([C2, nb], F32, tag="rv", name=f"rv{g}")
                for i, b in enumerate(bs):
                    nc.vector.reduce_max(
                        out=neg_mx[:, i:i + 1],
                        in_=kvt_red[:, b * C:(b + 1) * C],
                        axis=mybir.AxisListType.X,
                        negate=True,
                    )
                    nc.scalar.activation(
                        ex[:, i * C:(i + 1) * C],
                        kvt_red[:, b * C:(b + 1) * C],
                        mybir.ActivationFunctionType.Exp,
                        bias=neg_mx[:, i:i + 1], scale=1.0,
                        accum_out=sm[:, i:i + 1],
                    )
                    nc.vector.reciprocal(rv[:, i:i + 1], sm[:, i:i + 1])
                    nc.vector.tensor_scalar_mul(
                        ex[:, i * C:(i + 1) * C],
                        ex[:, i * C:(i + 1) * C],
                        rv[:, i:i + 1],
                    )
                    # Transpose softmaxed kvT to kv[c, d] (transpose-mode
                    # matmul must write PSUM partition 0); the PSUM->SBUF
                    # copies cast to fp16 for phase 2. The strip is copied
                    # twice — DVE writes partitions 0:64, ACT writes 64:128
                    # — so row-tiled phase-2 can read kv from the upper
                    # rows without a serializing SBUF->SBUF mirror DMA.
                    tp = ps.tile([C, C2], F32, tag="tp", name=f"tp{b}", bufs=2)
                    nc.tensor.transpose(
                        tp[:], ex[:, i * C:(i + 1) * C], ident[0:C2, 0:C2]
                    )
                    nc.vector.tensor_copy(
                        kv_soft[0:C, b * C2:(b + 1) * C2], tp[:]
                    )
                    nc.scalar.activation(
                        kv_soft[64:64 + C, b * C2:(b + 1) * C2],
                        tp[:],
                        mybir.ActivationFunctionType.Copy,
                    )
                # Phase 2, wide-moving form: out[d, n] = kv_soft^T @ kct
                # with kv_soft as the 64x64 stationary and kct as the moving
                # operand at the PE's max 512-wide free dim — 4 matmuls per
                # batch instead of 16 (the old form was LDWEIGHTS-dispatch
                # bound at ~151ns/instr). Row-half A (kct/kv rows 0:64,
                # tokens with n%16<8) runs on PE quadrant (0,0), half B on
                # (64,64) — concurrent subarrays — stacking both halves in
                # one [128, 512] psum bank per block. The transposed result
                # is stored as-is and un-permuted on the host.
                for b in bs:
                    o_ps = ps.tile(
                        [128, 2, 512], F32, tag="o", name=f"o{b}", bufs=2
                    )
                    for blk in range(2):
                        for hf in range(2):
                            r0 = 64 * hf
                            nc.tensor.matmul(
                                o_ps[r0:r0 + C2, blk, :],
                                lhsT=kv_soft[r0:r0 + C, b * C2:(b + 1) * C2],
                                rhs=kct_sb[
                                    r0:r0 + C,
                                    b * 1024 + blk * 512: b * 1024 + (blk + 1) * 512,
                                ],
                                start=True,
                                stop=True,
                                tile_position=(r0, r0),
                            )
                    stg = stage_pool.tile(
                        [128, (NSL // 128) * C2], F16, tag=f"stg{b}",
                        name=f"stg{b}",
                    )
                    nc.vector.tensor_add(
                        stg[:, 0:512],
                        o_ps[:, 0, :],
                        vc_sb[:, b * 1024: b * 1024 + 512],
                    )
                    # store the first block while the second block's add runs
                    nc.sync.dma_start(out[b][:, 0:512], stg[:, 0:512])
                    nc.vector.tensor_add(
                        stg[:, 512:1024],
                        o_ps[:, 1, :],
                        vc_sb[:, b * 1024 + 512: b * 1024 + 1024],
                    )
                    nc.sync.dma_start(out[b][:, 512:1024], stg[:, 512:1024])

            # ---- phase 1: partial kvT[d, c] per batch, col-tiled 2x ----
            # Plain fp8 matmuls (NOT DoubleRow: with free dim 64 DoubleRow
            # disables fast-weight-load and is a measured net loss). Even
            # token-tiles accumulate on PE column group 0 (psum rows 0:64),
            # odd tiles on column group 2 (psum rows 64:128); the two
            # groups' LDWEIGHTS/MATMUL overlap on independent subarrays.
            for b in range(N):
                kv_ps = ps.tile([128, C], F32, tag="kv", name=f"kv{b}")
                for h in range(2):
                    k_sb = big.tile([128, A_TILES, C], F8, tag="k")
                    v_sb = big.tile([128, A_TILES, C2], F8, tag="v")
                    nc.sync.dma_start(
                        k_sb[:], km[b, h].rearrange("p (a c) -> p a c", a=A_TILES)
                    )
                    nc.sync.dma_start(
                        v_sb[:], vm[b, h].rearrange("p (a c) -> p a c", a=A_TILES)
                    )
                    if h == 1:
                        # phase-2 inputs for batch b: issued on the scalar
                        # (ACT) DMA FIFO so they never delay the phase-1
                        # chunk stream on the sync FIFO.
                        nc.scalar.dma_start(
                            kct_sb[:, b * (NSL // 2):(b + 1) * (NSL // 2)],
                            kct[b],
                        )
                        nc.scalar.dma_start(
                            vc_sb[:, b * 1024:(b + 1) * 1024],
                            vc[b].rearrange("(p j) c -> p (j c)", p=128),
                        )
                    for a in range(A_TILES):
                        half = a % 2
                        nc.tensor.matmul(
                            kv_ps[64 * half:64 * half + C2, :],
                            lhsT=v_sb[:, a, :],
                            rhs=k_sb[:, a, :],
                            start=(h == 0 and a < 2),
                            stop=(h == 1 and a >= A_TILES - 2),
                            tile_position=(0, 64 * half),
                        )
                # partial kvT = even-half + odd-half (DVE can read only one
                # PSUM operand per instruction, so copy then add)
                nc.vector.tensor_copy(kvt_sb[:, b * C:(b + 1) * C], kv_ps[0:C2, :])
                nc.vector.tensor_add(
                    kvt_sb[:, b * C:(b + 1) * C],
                    kvt_sb[:, b * C:(b + 1) * C],
                    kv_ps[64:64 + C2, :],
                )
                # Uneven AllGather split (cheaper than AllReduce on the
                # CC core; the 8 partials are tree-reduced locally on DVE):
                # batches {0,1,2} gather as soon as b2's partial is ready —
                # the CC core's ~11us dequeue + transfer overlap the rest
                # of phase 1 and the b3 gather — and batch {3} gathers
                # alone (8KB in), so the serial post-collective tail covers
                # a single batch.
                if b in (2, 3):
                    g = b - 2
                    w = 3 * C if g == 0 else C
                    lo = 0 if g == 0 else 3 * C
                    ar_in = dram.tile(
                        [C2, w], F16, tag=f"ar_in{g}", name=f"ar_in{g}"
                    )
                    ar_out = dram.tile(
                        [N_CORES, C2, w], F16, addr_space="Shared",
                        tag=f"ar_out{g}", name=f"ar_out{g}",
                    )
                    ar_outs[g] = ar_out
                    nc.scalar.dma_start(ar_in[:], kvt_sb[:, lo:lo + w])
                    nc.gpsimd.collective_compute(
                        "AllGather",
                        mybir.AluOpType.bypass,
                        replica_groups=[list(range(N_CORES))],
                        ins=[ar_in.opt()],
                        outs=[ar_out.opt()],
                    )
            emit_tail((0, 1, 2), 0, kvt_allA)
            emit_tail((3,), 1, kvt_allB)

    nc.compile()
    return nc


def _get_program():
    if "nc" not in _CACHE:
        _CACHE["nc"] = _build_program()
    return _CACHE["nc"]


def kernel(key_mem, val_mem, key_cur, val_cur, alpha):
    key_mem = np.asarray(key_mem, dtype=np.float32)
    val_mem = np.asarray(val_mem, dtype=np.float32)
    key_cur = np.asarray(key_cur, dtype=np.float32)
    val_cur = np.asarray(val_cur, dtype=np.float32)
    alpha_f = float(np.asarray(alpha).reshape(-1)[0])

    nc = _get_program()

    # key_cur^T with alpha folded in; token axis permuted so that SBUF
    # column j*128+p holds token p*16+j (phase-2 store contiguity).
    kc_scaled = (alpha_f * key_cur).astype(np.float32)
    in_maps = []
    for i in range(N_CORES):
        kct_i = kc_scaled[:, i * NSL:(i + 1) * NSL, :].transpose(0, 2, 1)
        kct_i = (
            kct_i.reshape(N, C, 128, NSL // 128)
            .transpose(0, 1, 3, 2)
            .reshape(N, C, NSL)
        )
        # pack for row-tiled phase 2: rows 0:64 = tiles j=0..7,
        # rows 64:128 = tiles j=8..15
        kct_i = (
            kct_i.reshape(N, C, 2, NSL // 2)
            .transpose(0, 2, 1, 3)
            .reshape(N, 128, NSL // 2)
        )
        # phase-1 inputs: [b, h, p, a, c] with token t = h*8192 + p*64 + a
        # is a pure reshape of the [b, t, c] slice (p-major), so the DMA
        # sees dense 4KB partition rows.
        km_i = (
            key_mem[:, i]
            .reshape(N, 2, 128, A_TILES * C)
            .astype(ml_dtypes.float8_e4m3)
        )
        vm_i = (
            val_mem[:, i]
            .reshape(N, 2, 128, A_TILES * C2)
            .astype(ml_dtypes.float8_e4m3)
        )
        # val_cur packed to the phase-2 output layout [b, (half,d), (jm,p)]:
        # n = p*16 + half*8 + jm
        vc_i = val_cur[:, i * NSL:(i + 1) * NSL, :]
        vc_dn = (
            vc_i.reshape(N, 128, 2, 8, C2)
            .transpose(0, 2, 4, 3, 1)
            .reshape(N, 128, 1024)
        )
        in_maps.append(
            {
                "key_mem": np.ascontiguousarray(km_i),
                "val_mem": np.ascontiguousarray(vm_i),
                "key_curT": np.ascontiguousarray(kct_i.astype(np.float16)),
                "val_cur": np.ascontiguousarray(vc_dn.astype(np.float16)),
            }
        )

    res = bass_utils.run_bass_kernel_spmd(
        nc, in_maps, core_ids=list(range(N_CORES)), **_RUN_OPTS
    )
    _CACHE["last_result"] = res
    outs = []
    for i in range(N_CORES):
        # stored [b, (half,d), (jm,p)] -> [b, n, d] with n = p*16+half*8+jm
        st = res.results[i]["out"].reshape(N, 2, C2, 8, 128)
        outs.append(
            st.transpose(0, 4, 1, 3, 2).reshape(N, NSL, C2)
        )
    return np.concatenate(outs, axis=1).astype(np.float32)


# revision 16
# speedup vs baseline: 1.1676x; 1.1675x over previous
"""ChannelAttentionPropagation1D kernel for 8x TRN2 NeuronCores.

Reference computation (per batch b):
  kv[c,d]   = sum_{t,n} key_mem[b,t,n,c] * val_mem[b,t,n,d]    # (64, 64)
  kv_soft   = softmax(kv, axis=c)
  out[n,d]  = alpha * (key_cur[b] @ kv_soft)[n,d] + val_cur[b,n,d]

Sharding (8 cores):
  phase 1: core i contracts the t=i slice of key_mem/val_mem (16384 tokens
           per batch) into a partial kv^T, AllGathered over cores in two
           2-batch groups.
  phase 2: core i computes the n-slice [2048*i, 2048*(i+1)) of the output.

Precision: key_mem/val_mem are cast to fp8e4m3 and key_cur to fp16 on the
host; the output is stored fp16 and upcast on the host. Empirical rel-fro
error on the reference data is 6.8e-3, under the 2e-2 gate: the kv logits
have std ~600 so the softmax is near-one-hot, the fp8 matmuls accumulate
exactly in fp32 PSUM (4-bit mantissa products are exact), and the quant
noise on the logits (std ~30) flips an argmax only where the top-2 gap is
already tiny. fp8 halves the dominant HBM stream again vs fp16 and enables
DoubleRow matmuls (256-token contraction per PE instruction).

Layout notes:
  - phase 1 accumulates kvT[d,c] (PSUM) so the softmax axis c lands on the
    free axis; a tiny PE transpose afterwards yields kv_soft[c,d] (fp16).
  - all DRAM operands are host-packed to the exact SBUF layout so every DMA
    is a dense [128, rowbytes] block copy.
  - key_cur is transposed (and scaled by alpha) on the host so its channel
    axis is the SBUF partition axis; its token axis is permuted n = 16p + j
    so phase-2 output tiles assemble into contiguous-per-partition stores.
  - collectives: the framework barrier (device rendezvous) ends at
    ~skew+preamble; the CC core then serializes AllGathers at ~11us
    dequeue + transfer each. Two group AGs let the first one's transfer
    overlap the second half of phase 1, and group-major tails let the
    first group's phase 2 overlap the second AllGather.
"""

import numpy as np
import ml_dtypes

import concourse.bacc as bacc
import concourse.mybir as mybir
import concourse.tile as tile
from concourse import bass_utils, masks

F32 = mybir.dt.float32
F16 = mybir.dt.float16
F8 = mybir.dt.float8e4

N_CORES = 8
N, T, NTOK, C, C2 = 4, 8, 16384, 64, 64
NSL = NTOK // N_CORES  # 2048: phase-2 token slice per core
A_TILES = 64           # 128-token matmul tiles per half-batch chunk
HALF = NTOK // 2       # 8192 tokens per phase-1 DMA chunk

_CACHE = {}

# Extra kwargs forwarded to run_bass_kernel_spmd (used by the profiling
# harness to request an NTFF trace; empty for normal correctness runs).
_RUN_OPTS = {}


def _build_program():
    nc = bacc.Bacc(
        "TRN2",
        target_bir_lowering=False,
        debug=False,
        enable_asserts=False,
        num_devices=N_CORES,
    )

    # host-packed [b, h, p, a, c]: token t = h*8192 + p*64 + a
    km = nc.dram_tensor(
        "key_mem", [N, 2, 128, A_TILES * C], F8, kind="ExternalInput"
    ).ap()
    vm = nc.dram_tensor(
        "val_mem", [N, 2, 128, A_TILES * C2], F8, kind="ExternalInput"
    ).ap()
    # key_curT is host-packed [128, NSL/2]: rows 0:64 = channels for output
    # tiles j=0..7, rows 64:128 = channels for tiles j=8..15 (row-tiled
    # phase-2 pairs).
    kct = nc.dram_tensor(
        "key_curT", [N, 128, NSL // 2], F16, kind="ExternalInput"
    ).ap()
    vc = nc.dram_tensor("val_cur", [N, NSL, C2], F16, kind="ExternalInput").ap()
    # output is stored transposed-and-packed [b, (half,d), (jm,p)] and
    # un-permuted on the host (free); this lets phase 2 run with the
    # 512-wide moving operand (4 matmuls per batch instead of 16)
    out = nc.dram_tensor(
        "out", [N, 128, (NSL // 2)], F16, kind="ExternalOutput"
    ).ap()

    with tile.TileContext(nc) as tc:
        with (
            tc.tile_pool(name="persist", bufs=1) as persist,
            tc.tile_pool(name="big", bufs=4) as big,
            tc.tile_pool(name="tmp", bufs=2) as tmp,
            tc.tile_pool(name="stage", bufs=2) as stage_pool,
            tc.tile_pool(name="ps", bufs=2, space="PSUM") as ps,
            tc.tile_pool(name="dram", bufs=1, space="DRAM") as dram,
        ):
            ident = persist.tile([128, 128], F32)
            masks.make_identity(nc, ident[:])

            kct_sb = persist.tile([128, N * (NSL // 2)], F16)
            vc_sb = persist.tile([128, N * (NSL // 128) * C2], F16)

            kvt_sb = persist.tile([C2, N * C], F16)
            # AllGather results [d, (rank, group_width)], fp16 to halve
            # the collective payload and readback (partials are ~+-1500
            # with quant noise ~30 from fp8, so fp16 is free). Uneven
            # split: batches {0,1,2} gather first (their transfer and
            # tail overlap the second AG), batch {3} gathers alone so
            # the post-last-collective tail is minimal.
            kvt_allA = persist.tile([C2, N_CORES * 3 * C], F16)
            kvt_allB = persist.tile([C2, N_CORES * 1 * C], F16)
            kvt_red = persist.tile([C2, N * C], F16)
            kv_soft = persist.tile([128, N * C2], F16)
            ar_outs = {}

            def emit_tail(bs, g, kvt_all):
                """One group's AR readback + tree-reduce + softmax +
                transpose + phase 2 + stores, emitted after the whole
                phase 1 so a late peer can never block local phase-1 work
                (engine FIFOs run in program order). Group 0 ({0,1,2})
                overlaps group 1's tiny AllGather."""
                RW = len(bs) * C  # per-rank width in kvt_all
                # readback rides the scalar (ACT) DMA FIFO: the sync FIFO
                # carries the previous group's output stores at this point,
                # which would queue the readback behind them
                nc.scalar.dma_start(
                    kvt_all[:].rearrange("d (r c) -> d r c", r=N_CORES),
                    ar_outs[g].rearrange("r d c -> d r c"),
                )
                # tree-reduce the 8 ranks, then final add into kvt_red
                # (cols b*C of kvt_red match batch b)
                for width in (4 * RW, 2 * RW):
                    nc.vector.tensor_add(
                        kvt_all[:, 0:width],
                        kvt_all[:, 0:width],
                        kvt_all[:, width:2 * width],
                    )
                nc.vector.tensor_add(
                    kvt_red[:, bs[0] * C: bs[0] * C + RW],
                    kvt_all[:, 0:RW],
                    kvt_all[:, RW:2 * RW],
                )
                nb = len(bs)
                neg_mx = tmp.tile([C2, nb], F16, tag="mx", name=f"mx{g}")
                ex = tmp.tile([C2, nb * C], F32, tag="ex", name=f"ex{g}")
                sm = tmp.tile([C2, nb], F32, tag="sm", name=f"sm{g}")
                rv = tmp.tile([C2, nb], F32, tag="rv", name=f"rv{g}")
                for i, b in enumerate(bs):
                    nc.vector.reduce_max(
                        out=neg_mx[:, i:i + 1],
                        in_=kvt_red[:, b * C:(b + 1) * C],
                        axis=mybir.AxisListType.X,
                        negate=True,
                    )
                    nc.scalar.activation(
                        ex[:, i * C:(i + 1) * C],
                        kvt_red[:, b * C:(b + 1) * C],
                        mybir.ActivationFunctionType.Exp,
                        bias=neg_mx[:, i:i + 1], scale=1.0,
                        accum_out=sm[:, i:i + 1],
                    )
                    nc.vector.reciprocal(rv[:, i:i + 1], sm[:, i:i + 1])
                    nc.vector.tensor_scalar_mul(
                        ex[:, i * C:(i + 1) * C],
                        ex[:, i * C:(i + 1) * C],
                        rv[:, i:i + 1],
                    )
                    # Transpose softmaxed kvT to kv[c, d] (transpose-mode
                    # matmul must write PSUM partition 0); the PSUM->SBUF
                    # copies cast to fp16 for phase 2. The strip is copied
                    # twice — DVE writes partitions 0:64, ACT writes 64:128
                    # — so row-tiled phase-2 can read kv from the upper
                    # rows without a serializing SBUF->SBUF mirror DMA.
                    tp = ps.tile([C, C2], F32, tag="tp", name=f"tp{b}", bufs=2)
                    nc.tensor.transpose(
                        tp[:], ex[:, i * C:(i + 1) * C], ident[0:C2, 0:C2]
                    )
                    nc.vector.tensor_copy(
                        kv_soft[0:C, b * C2:(b + 1) * C2], tp[:]
                    )
                    nc.scalar.activation(
                        kv_soft[64:64 + C, b * C2:(b + 1) * C2],
                        tp[:],
                        mybir.ActivationFunctionType.Copy,
                    )
                # Phase 2, wide-moving form: out[d, n] = kv_soft^T @ kct
                # with kv_soft as the 64x64 stationary and kct as the moving
                # operand at the PE's max 512-wide free dim — 4 matmuls per
                # batch instead of 16 (the old form was LDWEIGHTS-dispatch
                # bound at ~151ns/instr). Row-half A (kct/kv rows 0:64,
                # tokens with n%16<8) runs on PE quadrant (0,0), half B on
                # (64,64) — concurrent subarrays — stacking both halves in
                # one [128, 512] psum bank per block. The transposed result
                # is stored as-is and un-permuted on the host.
                for b in bs:
                    o_ps = ps.tile(
                        [128, 2, 512], F32, tag="o", name=f"o{b}", bufs=2
                    )
                    for blk in range(2):
                        for hf in range(2):
                            r0 = 64 * hf
                            nc.tensor.matmul(
                                o_ps[r0:r0 + C2, blk, :],
                                lhsT=kv_soft[r0:r0 + C, b * C2:(b + 1) * C2],
                                rhs=kct_sb[
                                    r0:r0 + C,
                                    b * 1024 + blk * 512: b * 1024 + (blk + 1) * 512,
                                ],
                                start=True,
                                stop=True,
                                tile_position=(r0, r0),
                            )
                    stg = stage_pool.tile(
                        [128, (NSL // 128) * C2], F16, tag=f"stg{b}",
                        name=f"stg{b}",
                    )
                    nc.vector.tensor_add(
                        stg[:, 0:512],
                        o_ps[:, 0, :],
                        vc_sb[:, b * 1024: b * 1024 + 512],
                    )
                    # store the first block while the second block's add runs
                    nc.sync.dma_start(out[b][:, 0:512], stg[:, 0:512])
                    nc.vector.tensor_add(
                        stg[:, 512:1024],
                        o_ps[:, 1, :],
                        vc_sb[:, b * 1024 + 512: b * 1024 + 1024],
                    )
                    nc.sync.dma_start(out[b][:, 512:1024], stg[:, 512:1024])

            # ---- phase 1: partial kvT[d, c] per batch, col-tiled 2x ----
            # Plain fp8 matmuls (NOT DoubleRow: with free dim 64 DoubleRow
            # disables fast-weight-load and is a measured net loss). Even
            # token-tiles accumulate on PE column group 0 (psum rows 0:64),
            # odd tiles on column group 2 (psum rows 64:128); the two
            # groups' LDWEIGHTS/MATMUL overlap on independent subarrays.
            for b in range(N):
                kv_ps = ps.tile([128, C], F32, tag="kv", name=f"kv{b}")
                for h in range(2):
                    k_sb = big.tile([128, A_TILES, C], F8, tag="k")
                    v_sb = big.tile([128, A_TILES, C2], F8, tag="v")
                    nc.sync.dma_start(
                        k_sb[:], km[b, h].rearrange("p (a c) -> p a c", a=A_TILES)
                    )
                    nc.sync.dma_start(
                        v_sb[:], vm[b, h].rearrange("p (a c) -> p a c", a=A_TILES)
                    )
                    if h == 1:
                        # phase-2 inputs for batch b: issued on the scalar
                        # (ACT) DMA FIFO so they never delay the phase-1
                        # chunk stream on the sync FIFO.
                        nc.scalar.dma_start(
                            kct_sb[:, b * (NSL // 2):(b + 1) * (NSL // 2)],
                            kct[b],
                        )
                        nc.scalar.dma_start(
                            vc_sb[:, b * 1024:(b + 1) * 1024],
                            vc[b].rearrange("(p j) c -> p (j c)", p=128),
                        )
                    for a in range(A_TILES):
                        half = a % 2
                        nc.tensor.matmul(
                            kv_ps[64 * half:64 * half + C2, :],
                            lhsT=v_sb[:, a, :],
                            rhs=k_sb[:, a, :],
                            start=(h == 0 and a < 2),
                            stop=(h == 1 and a >= A_TILES - 2),
                            tile_position=(0, 64 * half),
                        )
                # partial kvT = even-half + odd-half (DVE can read only one
                # PSUM operand per instruction, so copy then add)
                nc.vector.tensor_copy(kvt_sb[:, b * C:(b + 1) * C], kv_ps[0:C2, :])
                nc.vector.tensor_add(
                    kvt_sb[:, b * C:(b + 1) * C],
                    kvt_sb[:, b * C:(b + 1) * C],
                    kv_ps[64:64 + C2, :],
                )
                # Uneven AllGather split (cheaper than AllReduce on the
                # CC core; the 8 partials are tree-reduced locally on DVE):
                # batches {0,1,2} gather as soon as b2's partial is ready —
                # the CC core's ~11us dequeue + transfer overlap the rest
                # of phase 1 and the b3 gather — and batch {3} gathers
                # alone (8KB in), so the serial post-collective tail covers
                # a single batch.
                if b in (2, 3):
                    g = b - 2
                    w = 3 * C if g == 0 else C
                    lo = 0 if g == 0 else 3 * C
                    ar_in = dram.tile(
                        [C2, w], F16, tag=f"ar_in{g}", name=f"ar_in{g}"
                    )
                    ar_out = dram.tile(
                        [N_CORES, C2, w], F16, addr_space="Shared",
                        tag=f"ar_out{g}", name=f"ar_out{g}",
                    )
                    ar_outs[g] = ar_out
                    nc.scalar.dma_start(ar_in[:], kvt_sb[:, lo:lo + w])
                    nc.gpsimd.collective_compute(
                        "AllGather",
                        mybir.AluOpType.bypass,
                        replica_groups=[list(range(N_CORES))],
                        ins=[ar_in.opt()],
                        outs=[ar_out.opt()],
                    )
            emit_tail((0, 1, 2), 0, kvt_allA)
            emit_tail((3,), 1, kvt_allB)

    nc.compile()
    return nc


def _get_program():
    if "nc" not in _CACHE:
        _CACHE["nc"] = _build_program()
    return _CACHE["nc"]


def kernel(key_mem, val_mem, key_cur, val_cur, alpha):
    key_mem = np.asarray(key_mem, dtype=np.float32)
    val_mem = np.asarray(val_mem, dtype=np.float32)
    key_cur = np.asarray(key_cur, dtype=np.float32)
    val_cur = np.asarray(val_cur, dtype=np.float32)
    alpha_f = float(np.asarray(alpha).reshape(-1)[0])

    nc = _get_program()

    # key_cur^T with alpha folded in; token axis permuted so that SBUF
    # column j*128+p holds token p*16+j (phase-2 store contiguity).
    kc_scaled = (alpha_f * key_cur).astype(np.float32)
    in_maps = []
    for i in range(N_CORES):
        kct_i = kc_scaled[:, i * NSL:(i + 1) * NSL, :].transpose(0, 2, 1)
        kct_i = (
            kct_i.reshape(N, C, 128, NSL // 128)
            .transpose(0, 1, 3, 2)
            .reshape(N, C, NSL)
        )
        # pack for row-tiled phase 2: rows 0:64 = tiles j=0..7,
        # rows 64:128 = tiles j=8..15
        kct_i = (
            kct_i.reshape(N, C, 2, NSL // 2)
            .transpose(0, 2, 1, 3)
            .reshape(N, 128, NSL // 2)
        )
        # phase-1 inputs: [b, h, p, a, c] with token t = h*8192 + p*64 + a
        # is a pure reshape of the [b, t, c] slice (p-major), so the DMA
        # sees dense 4KB partition rows.
        km_i = (
            key_mem[:, i]
            .reshape(N, 2, 128, A_TILES * C)
            .astype(ml_dtypes.float8_e4m3)
        )
        vm_i = (
            val_mem[:, i]
            .reshape(N, 2, 128, A_TILES * C2)
            .astype(ml_dtypes.float8_e4m3)
        )
        # val_cur packed to the phase-2 output layout [b, (half,d), (jm,p)]:
        # n = p*16 + half*8 + jm
        vc_i = val_cur[:, i * NSL:(i + 1) * NSL, :]
        vc_dn = (
            vc_i.reshape(N, 128, 2, 8, C2)
            .transpose(0, 2, 4, 3, 1)
            .reshape(N, 128, 1024)
        )
        in_maps.append(
            {
                "key_mem": np.ascontiguousarray(km_i),
                "val_mem": np.ascontiguousarray(vm_i),
                "key_curT": np.ascontiguousarray(kct_i.astype(np.float16)),
                "val_cur": np.ascontiguousarray(vc_dn.astype(np.float16)),
            }
        )

    res = bass_utils.run_bass_kernel_spmd(
        nc, in_maps, core_ids=list(range(N_CORES)), **_RUN_OPTS
    )
    _CACHE["last_result"] = res
    outs = []
    for i in range(N_CORES):
        # stored [b, (half,d), (jm,p)] -> [b, n, d] with n = p*16+half*8+jm
        st = res.results[i]["out"].reshape(N, 2, C2, 8, 128)
        outs.append(
            st.transpose(0, 4, 1, 3, 2).reshape(N, NSL, C2)
        )
    return np.concatenate(outs, axis=1).astype(np.float32)
